# revision 28
# baseline (speedup 1.0000x reference)
"""7x7 box blur (reflect padding, depthwise over channels) on TRN2, 8 cores.

Math: out = (1/49) * Bv^T @ X @ Bh per (batch, channel) image, where
Bv == Bh == B is the 512x512 banded 0/1/2 integer matrix encoding the
7-tap box window with reflect boundary folded in.  B is exact in fp16.

Two TensorE passes per image, no explicit transposes:
  pass 1: T1[w, h'] = sum_h X[h, w] * B[h, h']   (vertical blur, output
          transposed -- X block is the stationary lhsT operand)
  pass 2: O[h', w'] = sum_w T1[w, h'] * B[w, w'] (horizontal blur, output
          back in natural layout)

ilv=4 (default) layout: the host pre-permutes each image into 5 ragged
OVERLAPPING row-blocks (rows [0,128), [122,250), [244,372), [366,494),
[488,512)) so that every pass-1 output column h' has all 7 taps inside
exactly ONE block: each h' column is streamed through the PE exactly
once (512 streamed cols/set vs 1036 for the parity-interleaved r2
scheme), at the cost of +4.7% input bytes (24 duplicated rows).  The
host layout xm[i, p, b, w] also makes every load descriptor 4KB
(4 blocks x 512 fp16 contiguous per partition) and every store
descriptor 4KB (out[i, p, c, w], natural row 128c+p), quartering
DMA descriptor count vs the row-major layout.

Memory precision: both HBM sides are fp16 (host pre-casts, device
stores fp16, upcast on host).  Sharding: pure data parallel, batch dim
32 -> 8 cores x 4; each core does 12 images (4 batches x 3 channels).
"""

import numpy as np
from contextlib import ExitStack

H = W = 512
IMGS = 12          # images per core: 4 batches * 3 channels
N_CORES = 8
# output column windows (h' for pass 1, w' for pass 2)
WINS = [(0, 122), (122, 244), (244, 366), (366, 488), (488, 512)]
# input row-window of each pass-2 lhsT block (w range covering taps of WINS[j])
WBLK = [(0, 125), (119, 247), (241, 369), (363, 491), (485, 512)]
# pass-1 output windows (h'): three ~170-wide windows; each gets
# contributions from exactly two 128-row input blocks
P1_WINS = [(0, 170), (170, 340), (340, 512)]
# pass-1 matmul list: (input 128-row block b, output window index) in an order
# that keeps each PSUM write region homogeneous (write-then-accumulate)
P1_MMS = [(0, 0), (1, 0), (1, 1), (2, 1), (2, 2), (3, 2)]
P1_STRIDE = 172  # column stride of packed pass-1 rhs chunks

# --- r2-interleaved layout (ilv=2): partition p holds row pairs (2p, 2p+1),
# so every HBM<->SBUF descriptor covers 2 consecutive rows = 2 KB fp16,
# halving per-queue DMA descriptor-processing time.  Input h-blocks become
# (half s, parity r): rows 256*s + 2p + r.  Pass-1 h' regions: [0,259) from
# the s=0 blocks, [259,512) from s=1, plus a 6-wide boundary strip [253,259)
# accumulating the s=1 contribution on top of the s=0 result.
# entries: (s, r, a, b, start, stop).  The [259,512) group runs FIRST:
# start=True marks the whole 2KB PSUM zero-region pending-zero, so the
# second start must precede nothing it would invalidate -- after it, the
# [0,259) group's writes clear pending state over bytes the boundary
# strip then accumulates into.
R2_P1_MMS = [
    (1, 0, 259, 512, True, False),
    (1, 1, 259, 512, False, True),
    (0, 0, 0, 259, True, False),
    (0, 1, 0, 259, False, True),
    (1, 0, 253, 259, False, False),
    (1, 1, 253, 259, False, True),
]
R2_CHUNKS = [(0, 0), (0, 1), (1, 0), (1, 1)]  # pass-2 output (s, r) chunks

# --- ilv=4 ragged-overlap layout: 5 row-blocks, each output column h'
# owned by exactly one block (all 7 reflect-folded taps inside it).
P4_STARTS = [0, 122, 244, 366, 480]   # block first row
P4_K = [128, 128, 128, 128, 32]       # block row count (partitions)
P4_WIN = [(0, 125), (125, 247), (247, 369), (369, 491), (491, 512)]
# xv>=5: all w-blocks exactly 128 wide (FWL needs 128-col stationaries;
# K<128 matmuls measured ~2-4x slower per instruction).  Extra rows in
# each block carry zero weights in bh, so results are unchanged.
WBLK128 = [(0, 128), (119, 247), (241, 369), (363, 491), (384, 512)]

_STATE: dict = {}
LDW_OPT = False  # walrus ldw-opt pass is broken (codegen rejects standalone LDW)


def _patch_ldw():
    """Flip --enable-ldw-opt on the walrus command line (process-local)."""
    if not LDW_OPT or _STATE.get("ldw_patched"):
        return
    from concourse import bass_utils as _bu
    orig = _bu.run_command

    def run_command(argv, **kwargs):
        argv = ["--enable-ldw-opt=true" if a == "--enable-ldw-opt=false"
                else a for a in argv]
        return orig(argv, **kwargs)

    _bu.run_command = run_command
    _STATE["ldw_patched"] = True


def _band_matrix() -> np.ndarray:
    """B[i, j] = multiplicity of input row i among the 7 reflect-padded taps
    of output row j."""
    B = np.zeros((512, 512), np.float32)
    j = np.arange(512)
    for d in range(-3, 4):
        i = np.abs(j + d)
        i = np.where(i > 511, 1022 - i, i)
        np.add.at(B, (i, j), 1.0)
    return B


def _build_consts(ilv: int = 4, xv: int = 0) -> dict:
    import ml_dtypes
    B = _band_matrix()
    if ilv == 4:
        # pass-1 rhs: column j holds B[S_k(j) + p, j] for its owner block
        f8 = ml_dtypes.float8_e4m3
        bvp = np.zeros((128, 512), f8 if xv == 7 else np.float16)
        for S, K, (a, b) in zip(P4_STARTS, P4_K, P4_WIN):
            bvp[:K, a:b] = B[S:S + K, a:b]
        consts = {"bvp": bvp}
    elif ilv in (2, 3):
        # pass-1 rhs chunks packed tight in R2_P1_MMS order; block (s, r)
        # partition p maps to input row 256*s + 2p + r.
        width = sum(b - a for (_, _, a, b, _, _) in R2_P1_MMS)
        bv = np.zeros((128, width), np.float16)
        off = 0
        for (s, r, a, b, _, _) in R2_P1_MMS:
            bv[:, off:off + b - a] = B[256 * s + r: 256 * (s + 1): 2, a:b]
            off += b - a
        consts = {"bv": bv}
    else:
        # pass-1 rhs: for each (block b, window): rows 128b..128b+128 of B,
        # cols P1_WINS[win], padded to width P1_STRIDE, laid side by side.
        bv = np.zeros((128, P1_STRIDE * len(P1_MMS)), np.float16)
        for k, (b, win) in enumerate(P1_MMS):
            s, e = P1_WINS[win]
            bv[:, P1_STRIDE * k: P1_STRIDE * k + (e - s)] = \
                B[128 * b: 128 * (b + 1), s:e]
        consts = {"bv": bv}
    # pass-2 rhs: for window j, rows are remapped to block j's partition
    # space (partition p = global w row WBLK[j][0] + p), cols WINS[j].
    wblk = WBLK128 if xv >= 5 else WBLK
    bh = np.zeros((128, 512), np.float16)
    for jw, ((ws, we), (s, e)) in enumerate(zip(wblk, WINS)):
        bh[: we - ws, s:e] = B[ws:we, s:e]
    consts["bh"] = bh
    return consts


def _permute_in(xs: np.ndarray, fp8: bool = False
                ) -> tuple[np.ndarray, np.ndarray]:
    """[n, 512, 512] row-major -> (xm [n,128,4,512], xtl [n,32,512]).
    fp8: quantize x-0.5 to e4m3 (the +0.5 is restored on-device in the
    output bias), halving input bytes at ~7e-3 rel err."""
    import ml_dtypes
    dt = ml_dtypes.float8_e4m3 if fp8 else np.float16
    if fp8:
        xs = xs - np.float32(0.5)
    xm = np.stack([xs[:, S:S + 128] for S in P4_STARTS[:4]], axis=2)
    xtl = xs[:, 480:512]
    return (np.ascontiguousarray(xm).astype(dt),
            np.ascontiguousarray(xtl).astype(dt))


def _unpermute_out(o: np.ndarray) -> np.ndarray:
    """Device output -> [n, 512, 512] natural layout.
    [n,128,4,512] chunk layout (row 128c+p at [:, p, c]) or [n,512,512]
    transposed (xv=6)."""
    if o.ndim == 4:
        return o.transpose(0, 2, 1, 3).reshape(o.shape[0], 512, 512)
    return np.ascontiguousarray(o.transpose(0, 2, 1))


def _build_nc(repeat: int = 1, loop_repeat: int = 0, group: int = -1,
              variant: str = "full", load_eng: str = "gp",
              store_eng: str = "rot", balance: int = 1, deep: int = 0,
              ilv: int = 4, pb: int = 53, xv: int = 7):
    """loop_repeat > 0 wraps the whole 12-image pipeline in a runtime
    For_i loop executing it that many times -- used only for timing (one
    NEFF dispatch, loop_repeat x the device work).
    variant: 'full' | 'dma' (loads+stores only) | 'nostore' (loads+compute)
    | 'load' (loads only) | 'store' (stores only) -- diagnostic builds.
    load_eng / store_eng: which queue issues the transfer --
    'gp' (SWDGE), 'sp' (qSP HWDGE), 'act' (qActivation HWDGE)."""
    _patch_ldw()
    do_dma = variant in ("full", "dma", "nostore", "load")
    do_compute = variant in ("full", "nostore", "comp")
    do_store = variant in ("full", "dma", "store")
    import concourse.tile as tile
    from concourse import bacc, mybir

    f16 = mybir.dt.float16
    f32 = mybir.dt.float32
    # xv=7: fp8 e4m3 input path (host sends x-0.5 quantized; +0.5 restored
    # in the output scale bias); halves input HBM traffic
    fx = mybir.dt.float8e4 if (ilv == 4 and xv == 7) else f16

    nc = bacc.Bacc("TRN2", target_bir_lowering=False, debug=False,
                   enable_asserts=True)
    IW = 5 * W if ilv == 4 else 4 * W   # per-image SBUF input width
    OW = 5 * W if (ilv == 4 and xv == 6) else 4 * W  # per-image output width

    if ilv == 4:
        xm_ap = nc.dram_tensor("xm", [IMGS, 128, 4, W], fx,
                               kind="ExternalInput").ap()
        xtl_ap = nc.dram_tensor("xtl", [IMGS, 32, W], fx,
                                kind="ExternalInput").ap()
        bvp_ap = nc.dram_tensor("bvp", [128, 512], fx,
                                kind="ExternalInput").ap()
        if xv == 6:
            # transposed output [i, w', h'] -- host untransposes
            out_ap = nc.dram_tensor("out", [IMGS, W, H], f16,
                                    kind="ExternalOutput").ap()
        else:
            out_ap = nc.dram_tensor("out", [IMGS, 128, 4, W], f16,
                                    kind="ExternalOutput").ap()
    else:
        bv_cols = (sum(b - a for (_, _, a, b, _, _) in R2_P1_MMS)
                   if ilv in (2, 3) else P1_STRIDE * len(P1_MMS))
        x_ap = nc.dram_tensor("x", [IMGS, H, W], f16,
                              kind="ExternalInput").ap()
        bv_ap = nc.dram_tensor("bv", [128, bv_cols], f16,
                               kind="ExternalInput").ap()
        out_ap = nc.dram_tensor("out", [IMGS, H, W], f16,
                                kind="ExternalOutput").ap()
    bh_ap = nc.dram_tensor("bh", [128, 512], f16, kind="ExternalInput").ap()

    def io_view(ap, gstart, GROUP, r2):
        # HBM <-> SBUF view of a group of images; r2 pairs 2 consecutive
        # rows per partition, merged into one 1024-elem (2 KB) run so each
        # DMA descriptor covers both rows.
        if r2:
            return ap[gstart:gstart + GROUP].rearrange(
                "i (s p r) w -> p i s (r w)", s=2, p=128, r=2)
        return ap[gstart:gstart + GROUP].rearrange(
            "i (s p) w -> p i s w", p=128)

    def out_view4(gstart, GROUP):
        return out_ap[gstart:gstart + GROUP].rearrange(
            "i p c w -> p i (c w)")

    ld_r2 = ilv in (2, 3)   # input layout / pass-1 block structure
    st_r2 = ilv == 2        # output layout / pass-2 chunk structure

    def dma_eng(which):
        return {"gp": nc.gpsimd, "sp": nc.sync, "act": nc.scalar}[which]

    def split_engs(spec):
        return ((spec[:2], spec[2:]) if spec[:2] in ("gp", "sp")
                else (spec[:3], spec[3:]))

    with tile.TileContext(nc) as tc, ExitStack() as ctx:
        cpool = ctx.enter_context(tc.tile_pool(name="const", bufs=1))
        xpool = ctx.enter_context(
            tc.tile_pool(name="xin", bufs=5 if deep else 3))
        t1ppool = ctx.enter_context(
            tc.tile_pool(name="t1p", bufs=pb // 10, space="PSUM"))
        t1pool = ctx.enter_context(tc.tile_pool(name="t1", bufs=10))
        oppool = ctx.enter_context(
            tc.tile_pool(name="opsum", bufs=pb % 10, space="PSUM"))
        outpool = ctx.enter_context(
            tc.tile_pool(name="osb", bufs=4 if deep else 3))

        if ilv == 4:
            bv = cpool.tile([128, 512], fx)
            nc.sync.dma_start(bv[:], bvp_ap[:])
        else:
            bv = cpool.tile([128, bv_cols], f16)
            nc.sync.dma_start(bv[:], bv_ap[:])
        bh = cpool.tile([128, 512], f16)
        nc.sync.dma_start(bh[:], bh_ap[:])

        dummy_osb = None
        if variant in ("dma", "store"):
            dummy_osb = cpool.tile([128, group * OW], f16)
            nc.vector.memset(dummy_osb[:], 0.0)
        dummy_x = None
        if variant == "comp":
            # compute-only: all groups read one zeroed const tile
            dummy_x = cpool.tile([128, max(2, abs(group)) * IW], fx)
            nc.vector.memset(dummy_x[:], 0.0)
        loop_ctx = (tc.For_i(0, loop_repeat, 1,
                             hint_engines=(mybir.EngineType.PE,))
                    if loop_repeat > 0 else None)
        if loop_ctx is not None:
            ctx.enter_context(loop_ctx)
        if group == 0:
            sizes = [1, 1, 2, 2, 2, 2, 1, 1]
        elif group == -1:
            sizes = [1, 1, 1, 2, 2, 2, 1, 1, 1]
        else:
            sizes = [group] * (IMGS // group)
        assert sum(sizes) == IMGS
        sched = []
        for rep in range(repeat):
            s0 = 0
            for gnum, sz in enumerate(sizes):
                for gi in range(sz):
                    sched.append((s0, sz, gi, gnum))
                s0 += sz
        ROT = ["gp", "sp", "act"]

        def load_group(xtg, gstart, GROUP, gnum):
            if ilv == 4:
                xgv = xtg.rearrange("p (i f) -> p i f", f=IW)
                dstm = xgv[:, :, :4 * W]
                dstt = xgv[:32, :, 4 * W:]
                srcm = xm_ap[gstart:gstart + GROUP].rearrange(
                    "i p b w -> p i (b w)")
                srct = xtl_ap[gstart:gstart + GROUP].rearrange(
                    "i p w -> p i w")
                if load_eng == "rot":
                    e = dma_eng(ROT[gnum % 3])
                    e.dma_start(dstm, srcm)
                    e.dma_start(dstt, srct)
                elif len(load_eng) > 3:
                    e0, e1 = split_engs(load_eng)
                    dma_eng(e0).dma_start(dstm[:64], srcm[:64])
                    dma_eng(e1).dma_start(dstm[64:], srcm[64:])
                    dma_eng(e1).dma_start(dstt, srct)
                else:
                    e = dma_eng(load_eng)
                    e.dma_start(dstm, srcm)
                    e.dma_start(dstt, srct)
                return
            src = io_view(x_ap, gstart, GROUP, ld_r2)
            if load_eng == "rot":
                dma_eng(ROT[gnum % 3]).dma_start(xtg[:], src)
            elif len(load_eng) > 3:  # e.g. 'gpsp', 'gpact', 'spact'
                e0, e1 = split_engs(load_eng)
                smid = 1 if ld_r2 else 2
                for i in range(GROUP):
                    dma_eng(e0).dma_start(
                        xtg[:, i * 4 * W: i * 4 * W + 2 * W],
                        src[:, i, :smid])
                    dma_eng(e1).dma_start(
                        xtg[:, i * 4 * W + 2 * W: (i + 1) * 4 * W],
                        src[:, i, smid:])
            else:
                dma_eng(load_eng).dma_start(xtg[:], src)

        def store_group6(osbg, gstart, GROUP, gnum):
            # transposed store: per w'-window jw, partitions = w' - s
            if store_eng == "rot":
                engs = [dma_eng(ROT[(gnum + 1 + jw) % 3]) for jw in range(5)]
            elif len(store_eng) > 3:
                e0, e1 = split_engs(store_eng)
                engs = [dma_eng(e0), dma_eng(e1), dma_eng(e0), dma_eng(e1),
                        dma_eng(e0)]
            else:
                engs = [dma_eng(store_eng)] * 5
            ov = osbg.rearrange("p (i f) -> p i f", f=5 * W)
            for jw, (s, e) in enumerate(WINS):
                dst = out_ap[gstart:gstart + GROUP, s:e].rearrange(
                    "i p h -> p i h")
                engs[jw].dma_start(dst, ov[:e - s, :, jw * W:(jw + 1) * W])

        def store_group(osbg, gstart, GROUP, gnum):
            if ilv == 4 and xv == 6:
                store_group6(osbg, gstart, GROUP, gnum)
                return
            out_view = (out_view4(gstart, GROUP) if ilv == 4
                        else io_view(out_ap, gstart, GROUP, st_r2))
            if store_eng == "rot":
                dma_eng(ROT[(gnum + 1) % 3]).dma_start(
                    out_view, osbg[:, :GROUP * 4 * W])
            elif len(store_eng) > 3:
                e0, e1 = split_engs(store_eng)
                if ilv == 4:
                    ov = out_view
                    src = osbg[:, :GROUP * 4 * W]
                    dma_eng(e0).dma_start(ov[:64], src[:64])
                    dma_eng(e1).dma_start(ov[64:], src[64:])
                else:
                    smid = 1 if st_r2 else 2
                    for i in range(GROUP):
                        dma_eng(e0).dma_start(
                            out_view[:, i, :smid],
                            osbg[:, i * 4 * W: i * 4 * W + 2 * W])
                        dma_eng(e1).dma_start(
                            out_view[:, i, smid:],
                            osbg[:, i * 4 * W + 2 * W: (i + 1) * 4 * W])
            else:
                dma_eng(store_eng).dma_start(
                    out_view, osbg[:, :GROUP * 4 * W])

        for (gstart, GROUP, g, gnum) in sched:
            img = gstart + g
            if g == 0:
                # load one image group; ilv=4 layout:
                # xtg[:, IW*i + 512*b + w] = block b of image gstart+i
                if variant == "comp":
                    xtg = dummy_x[:, :GROUP * IW]
                else:
                    xtg = xpool.tile([128, max(2, abs(group)) * IW], fx,
                                     tag="xt")
                    xtg = xtg[:, :GROUP * IW]
                if do_dma:
                    load_group(xtg, gstart, GROUP, gnum)
                if ilv == 4 and xv >= 5 and do_compute and variant != "comp":
                    # zero-pad the 24-row tail block to K=128 so its mms
                    # run as regular (128,128) tiles (K<128 mms are slow)
                    for i in range(GROUP):
                        eng = nc.vector if (gstart + i) % 2 == 0 else nc.gpsimd
                        # engine partition windows must be aligned pow2 blocks
                        eng.memset(
                            xtg[32:64, i * IW + 4 * W:(i + 1) * IW], 0.0)
                        eng.memset(
                            xtg[64:, i * IW + 4 * W:(i + 1) * IW], 0.0)
            xt = xtg[:, g * IW:(g + 1) * IW]
            if not do_compute:
                if g == GROUP - 1 and do_store:
                    store_group(dummy_osb, gstart, GROUP, gnum)
                continue

            # pass 1: T1[w, h'] per overlapping w-block j
            t1_tiles = []
            if ilv == 4 and xv == 4:
                # batched-tail order: all 5 K=24 tail mms first (one PE
                # tile-size reconfig pair per image, not per set)
                t1ps = []
                for jw, (ws, we) in enumerate(WBLK):
                    mj = we - ws
                    t1p = t1ppool.tile([128, 512], f32, tag="t1p")
                    a, b = P4_WIN[4]
                    nc.tensor.matmul(
                        t1p[:mj, a:b],
                        lhsT=xt[:P4_K[4], 4 * W + ws: 4 * W + we],
                        rhs=bv[:P4_K[4], a:b],
                        start=True, stop=False,
                    )
                    t1ps.append(t1p)
                for jw, (ws, we) in enumerate(WBLK):
                    mj = we - ws
                    t1p = t1ps[jw]
                    for k in range(4):
                        a, b = P4_WIN[k]
                        nc.tensor.matmul(
                            t1p[:mj, a:b],
                            lhsT=xt[:128, k * W + ws: k * W + we],
                            rhs=bv[:128, a:b],
                            start=False, stop=(k == 3),
                        )
                    t1 = t1pool.tile([128, 512], f16, tag="t1")
                    if balance == 3 and (jw + img) % 3 == 2:
                        nc.gpsimd.tensor_scalar_mul(t1[:mj, :], t1p[:mj, :], 1.0)
                    elif balance == 3 and (jw + img) % 3 == 0:
                        nc.vector.tensor_scalar_mul(t1[:mj, :], t1p[:mj, :], 1.0)
                    elif balance and (jw + img) % 2 == 0:
                        nc.vector.tensor_scalar_mul(t1[:mj, :], t1p[:mj, :], 1.0)
                    else:
                        nc.scalar.copy(t1[:mj, :], t1p[:mj, :])
                    t1_tiles.append((t1, mj))
            wblk = WBLK128 if (ilv == 4 and xv >= 5) else WBLK
            for jw, (ws, we) in (
                    [] if (ilv == 4 and xv == 4) else list(enumerate(wblk))):
                mj = we - ws
                t1p = t1ppool.tile([128, 512], f32, tag="t1p")
                if ilv == 4:
                    nk = 4 if xv in (1, 3) else 5
                    for k in range(nk):
                        a, b = P4_WIN[k]
                        K = 128 if xv >= 5 else P4_K[k]
                        nc.tensor.matmul(
                            t1p[:mj, a:b],
                            lhsT=xt[:K, k * W + ws: k * W + we],
                            rhs=bv[:K, a:b],
                            start=(k == 0), stop=(k == nk - 1),
                        )
                elif ld_r2:
                    off = 0
                    for (s, r, a, b, st, sp) in R2_P1_MMS:
                        nc.tensor.matmul(
                            t1p[:mj, a:b],
                            lhsT=xt[:, (s * 2 + r) * W + ws:
                                    (s * 2 + r) * W + we],
                            rhs=bv[:, off:off + b - a],
                            start=st, stop=sp,
                            # the boundary-strip mms re-open [253,259) to
                            # accumulate the s=1 rows onto the closed s=0
                            # group; plain PSUM read-modify-write on HW
                            skip_group_check=(a, b) == (253, 259) and not st,
                        )
                        off += b - a
                else:
                    for k, (b, win) in enumerate(P1_MMS):
                        s, e = P1_WINS[win]
                        nc.tensor.matmul(
                            t1p[:mj, s:e],
                            lhsT=xt[:, b * W + ws: b * W + we],
                            rhs=bv[:, P1_STRIDE * k: P1_STRIDE * k + (e - s)],
                            start=(k == 0), stop=(k == len(P1_MMS) - 1),
                        )
                t1 = t1pool.tile([128, 512], f16, tag="t1")
                # PSUM->SBUF fp16 cast, alternating ACT/DVE to balance load
                if balance == 3 and (jw + img) % 3 == 2:
                    nc.gpsimd.tensor_scalar_mul(t1[:mj, :], t1p[:mj, :], 1.0)
                elif balance == 3 and (jw + img) % 3 == 0:
                    nc.vector.tensor_scalar_mul(t1[:mj, :], t1p[:mj, :], 1.0)
                elif balance and (jw + img) % 2 == 0:
                    nc.vector.tensor_scalar_mul(t1[:mj, :], t1p[:mj, :], 1.0)
                else:
                    nc.scalar.copy(t1[:mj, :], t1p[:mj, :])
                t1_tiles.append((t1, mj))

            # pass 2: O[h', w'] per 128-row h' chunk c
            if g == 0:
                osbg = outpool.tile([128, max(2, abs(group)) * OW], f16,
                                    tag="osb")
                osbg = osbg[:, :GROUP * OW]
            osb = osbg[:, g * OW:(g + 1) * OW]
            if ilv == 4 and xv == 6:
                # flipped pass 2: stationary = bh block, moving = full t1;
                # one N=512 matmul per w-block, output transposed [w', h']
                for jw, (t1, mj) in enumerate(t1_tiles):
                    s, e = WINS[jw]
                    opt = oppool.tile([128, 512], f32, tag="op")
                    nc.tensor.matmul(
                        opt[:e - s, :],
                        lhsT=bh[:mj, s:e],
                        rhs=t1[:mj, :],
                        start=True, stop=True,
                    )
                    dst = osb[:, jw * W:(jw + 1) * W]
                    if (jw + img) % 2 == 0:
                        nc.vector.tensor_scalar_mul(
                            dst[:e - s], opt[:e - s, :], 1.0 / 49.0)
                    else:
                        nc.scalar.mul(dst[:e - s], opt[:e - s, :], 1.0 / 49.0)
                if g == GROUP - 1 and do_store:
                    store_group(osbg, gstart, GROUP, gnum)
                continue
            for c in range(4):
                op = oppool.tile([128, 512], f32, tag="op")
                p2 = t1_tiles[:4] if xv in (2, 3) else t1_tiles
                for jw, (t1, mj) in enumerate(p2):
                    s, e = WINS[jw]
                    if st_r2:
                        # chunk c=(sc, rc): output rows 256*sc + 2p + rc,
                        # i.e. t1 cols strided 2 starting at 256*sc + rc
                        sc, rc = R2_CHUNKS[c]
                        lhsT = t1[:mj].rearrange(
                            "q (s p r) -> q s r p", s=2, p=128, r=2)[:, sc, rc, :]
                    else:
                        lhsT = t1[:mj, c * 128: (c + 1) * 128]
                    nc.tensor.matmul(
                        op[:, s:e],
                        lhsT=lhsT,
                        rhs=bh[:mj, s:e],
                        start=(jw == 0), stop=(jw == len(p2) - 1),
                    )
                # final 1/49 scale + PSUM->SBUF fp16, split across DVE and ACT
                dst = osb[:, c * W: (c + 1) * W]
                bias = 0.5 if xv == 7 else 0.0
                if c % 2 == 0:
                    if bias:
                        nc.vector.tensor_scalar(
                            dst, op[:], 1.0 / 49.0, bias,
                            mybir.AluOpType.mult, mybir.AluOpType.add)
                    else:
                        nc.vector.tensor_scalar_mul(dst, op[:], 1.0 / 49.0)
                else:
                    nc.scalar.activation(
                        dst, op[:], mybir.ActivationFunctionType.Copy,
                        bias=bias, scale=1.0 / 49.0)
            if g == GROUP - 1 and do_store:
                store_group(osbg, gstart, GROUP, gnum)

    nc.compile()
    return nc


def _get_state(repeat: int = 1, loop_repeat: int = 0, group: int = -1,
               variant: str = "full", load_eng: str = "gp",
               store_eng: str = "rot", balance: int = 1, deep: int = 0,
               ilv: int = 4, pb: int = 53, xv: int = 7):
    key = ("nc", repeat, loop_repeat, group, variant, load_eng, store_eng,
           balance, deep, ilv, pb, xv)
    if key not in _STATE:
        _STATE[key] = _build_nc(repeat, loop_repeat, group, variant,
                                load_eng, store_eng, balance, deep, ilv, pb,
                                xv)
    ckey = ("consts", ilv, xv)
    if ckey not in _STATE:
        _STATE[ckey] = _build_consts(ilv, xv)
    return {"nc": _STATE[key], "consts": _STATE[ckey], "ilv": ilv, "xv": xv}


def _make_runner(repeat: int = 1, loop_repeat: int = 0, group: int = -1,
                 variant: str = "full", load_eng: str = "gp",
                 store_eng: str = "rot", balance: int = 1, deep: int = 0,
                 ilv: int = 4, pb: int = 53, xv: int = 7):
    """Cached 8-core sharded jit over the bass program (mirrors
    bass2jax.run_bass_via_pjrt's multicore path, minus buffer donation so
    the compiled fn can be invoked repeatedly for timing)."""
    rkey = ("runner", repeat, loop_repeat, group, variant, load_eng,
            store_eng, balance, deep, ilv, pb, xv)
    if rkey in _STATE:
        return _STATE[rkey]
    import jax
    import jax.numpy as jnp
    from jax.sharding import Mesh, PartitionSpec
    from jax.experimental.shard_map import shard_map
    from concourse import bass2jax, mybir

    st = _get_state(repeat, loop_repeat, group, variant, load_eng,
                    store_eng, balance, deep, ilv, pb, xv)
    nc = st["nc"]
    bass2jax.install_neuronx_cc_hook()

    partition_name = (nc.partition_id_tensor.name
                      if nc.partition_id_tensor else None)
    in_names, out_names, out_avals = [], [], []
    for alloc in nc.m.functions[0].allocations:
        if not isinstance(alloc, mybir.MemoryLocationSet):
            continue
        name = alloc.memorylocations[0].name
        if alloc.kind == "ExternalInput":
            if name != partition_name:
                in_names.append(name)
        elif alloc.kind == "ExternalOutput":
            out_names.append(name)
            out_avals.append(jax.core.ShapedArray(
                tuple(alloc.tensor_shape), mybir.dt.np(alloc.dtype)))
    n_params = len(in_names)
    all_names = in_names + out_names
    if partition_name is not None:
        all_names = all_names + [partition_name]

    def _body(*args):
        operands = list(args)
        if partition_name is not None:
            operands.append(bass2jax.partition_id_tensor())
        outs = bass2jax._bass_exec_p.bind(
            *operands,
            out_avals=tuple(out_avals),
            in_names=tuple(all_names),
            out_names=tuple(out_names),
            lowering_input_output_aliases=(),
            sim_require_finite=True,
            sim_require_nnan=True,
            nc=nc,
        )
        return tuple(outs)

    devices = jax.devices()[:N_CORES]
    mesh = Mesh(np.asarray(devices), ("core",))
    n_outs = len(out_names)
    sharded = jax.jit(shard_map(
        _body, mesh=mesh,
        in_specs=(PartitionSpec("core"),) * (n_params + n_outs),
        out_specs=(PartitionSpec("core"),) * n_outs,
        check_rep=False))
    _STATE[rkey] = (sharded, in_names, out_names, out_avals)
    return _STATE[rkey]


def _core_inputs(x: np.ndarray, ilv: int, consts: dict,
                 xv: int = 0) -> list[dict]:
    """Per-core input dicts from the full fp32 batch."""
    B, C = x.shape[0], x.shape[1]
    per = B // N_CORES
    maps = []
    for i in range(N_CORES):
        xs = np.ascontiguousarray(
            x[i * per:(i + 1) * per].reshape(per * C, H, W))
        if ilv == 4:
            xm, xtl = _permute_in(xs, fp8=(xv == 7))
            m = {"xm": xm, "xtl": xtl}
        else:
            m = {"x": xs.astype(np.float16)}
        m.update(consts)
        maps.append(m)
    return maps


def _concat_inputs(x: np.ndarray, **kw):
    st = _get_state(**kw)
    _, in_names, out_names, out_avals = _make_runner(**kw)
    maps = _core_inputs(np.asarray(x, np.float32), st["ilv"], st["consts"],
                        st["xv"])
    concat_in = []
    for n in in_names:
        a = np.stack([m[n] for m in maps])
        concat_in.append(np.ascontiguousarray(
            a.reshape((N_CORES * a.shape[1],) + a.shape[2:])))
    concat_zeros = [
        np.zeros((N_CORES * a.shape[0],) + a.shape[1:], a.dtype)
        for a in out_avals]
    return concat_in, concat_zeros


def kernel(x: np.ndarray) -> np.ndarray:
    _patch_ldw()
    from concourse import bass_utils
    st = _get_state()
    x = np.asarray(x, np.float32)
    B, C = x.shape[0], x.shape[1]
    per = B // N_CORES
    in_maps = _core_inputs(x, st["ilv"], st["consts"], st["xv"])
    res = bass_utils.run_bass_kernel_spmd(
        st["nc"], in_maps, core_ids=list(range(N_CORES)))
    outs = []
    for i in range(N_CORES):
        o = res.results[i]["out"]
        if st["ilv"] == 4:
            o = _unpermute_out(np.asarray(o))
        outs.append(o.reshape(per, C, H, W))
    out = np.concatenate(outs, axis=0)
    return np.ascontiguousarray(out).astype(np.float32)


def benchmark(x: np.ndarray, iters: int = 30) -> float:
    """Returns steady-state per-invocation wall time in ns for the 8-core
    SPMD execution (inputs sharded and resident on their devices; outputs
    chained into the next call's scratch operand so iterations pipeline
    without host round-trips)."""
    import time
    import jax
    from jax.sharding import Mesh, NamedSharding, PartitionSpec

    x = np.asarray(x, np.float32)
    sharded, in_names, out_names, out_avals = _make_runner()
    concat_in, concat_zeros = _concat_inputs(x)
    devices = jax.devices()[:N_CORES]
    mesh = Mesh(np.asarray(devices), ("core",))
    shard0 = NamedSharding(mesh, PartitionSpec("core"))
    dev_in = [jax.device_put(a, shard0) for a in concat_in]
    dev_zero = [jax.device_put(a, shard0) for a in concat_zeros]
    # warm up (compiles on first call)
    outs = sharded(*dev_in, *dev_zero)
    jax.block_until_ready(outs)
    # chained steady-state loop: prior outputs feed the scratch-out slots
    t0 = time.perf_counter()
    for _ in range(iters):
        outs = sharded(*dev_in, *outs)
    jax.block_until_ready(outs)
    dt = (time.perf_counter() - t0) / iters
    return dt * 1e9


# revision 29
# speedup vs baseline: 1.0292x; 1.0292x over previous
"""7x7 box blur (reflect padding, depthwise over channels) on TRN2, 8 cores.

Math: out = (1/49) * Bv^T @ X @ Bh per (batch, channel) image, where
Bv == Bh == B is the 512x512 banded 0/1/2 integer matrix encoding the
7-tap box window with reflect boundary folded in.  B is exact in fp16.

Two TensorE passes per image, no explicit transposes:
  pass 1: T1[w, h'] = sum_h X[h, w] * B[h, h']   (vertical blur, output
          transposed -- X block is the stationary lhsT operand)
  pass 2: O[h', w'] = sum_w T1[w, h'] * B[w, w'] (horizontal blur, output
          back in natural layout)

ilv=4 (default) layout: the host pre-permutes each image into 5 ragged
OVERLAPPING row-blocks (rows [0,128), [122,250), [244,372), [366,494),
[488,512)) so that every pass-1 output column h' has all 7 taps inside
exactly ONE block: each h' column is streamed through the PE exactly
once (512 streamed cols/set vs 1036 for the parity-interleaved r2
scheme), at the cost of +4.7% input bytes (24 duplicated rows).  The
host layout xm[i, p, b, w] also makes every load descriptor 4KB
(4 blocks x 512 fp16 contiguous per partition) and every store
descriptor 4KB (out[i, p, c, w], natural row 128c+p), quartering
DMA descriptor count vs the row-major layout.

Memory precision: both HBM sides are fp16 (host pre-casts, device
stores fp16, upcast on host).  Sharding: pure data parallel, batch dim
32 -> 8 cores x 4; each core does 12 images (4 batches x 3 channels).
"""

import numpy as np
from contextlib import ExitStack

H = W = 512
IMGS = 12          # images per core: 4 batches * 3 channels
N_CORES = 8
# output column windows (h' for pass 1, w' for pass 2)
WINS = [(0, 122), (122, 244), (244, 366), (366, 488), (488, 512)]
# input row-window of each pass-2 lhsT block (w range covering taps of WINS[j])
WBLK = [(0, 125), (119, 247), (241, 369), (363, 491), (485, 512)]
# pass-1 output windows (h'): three ~170-wide windows; each gets
# contributions from exactly two 128-row input blocks
P1_WINS = [(0, 170), (170, 340), (340, 512)]
# pass-1 matmul list: (input 128-row block b, output window index) in an order
# that keeps each PSUM write region homogeneous (write-then-accumulate)
P1_MMS = [(0, 0), (1, 0), (1, 1), (2, 1), (2, 2), (3, 2)]
P1_STRIDE = 172  # column stride of packed pass-1 rhs chunks

# --- r2-interleaved layout (ilv=2): partition p holds row pairs (2p, 2p+1),
# so every HBM<->SBUF descriptor covers 2 consecutive rows = 2 KB fp16,
# halving per-queue DMA descriptor-processing time.  Input h-blocks become
# (half s, parity r): rows 256*s + 2p + r.  Pass-1 h' regions: [0,259) from
# the s=0 blocks, [259,512) from s=1, plus a 6-wide boundary strip [253,259)
# accumulating the s=1 contribution on top of the s=0 result.
# entries: (s, r, a, b, start, stop).  The [259,512) group runs FIRST:
# start=True marks the whole 2KB PSUM zero-region pending-zero, so the
# second start must precede nothing it would invalidate -- after it, the
# [0,259) group's writes clear pending state over bytes the boundary
# strip then accumulates into.
R2_P1_MMS = [
    (1, 0, 259, 512, True, False),
    (1, 1, 259, 512, False, True),
    (0, 0, 0, 259, True, False),
    (0, 1, 0, 259, False, True),
    (1, 0, 253, 259, False, False),
    (1, 1, 253, 259, False, True),
]
R2_CHUNKS = [(0, 0), (0, 1), (1, 0), (1, 1)]  # pass-2 output (s, r) chunks

# --- ilv=4 ragged-overlap layout: 5 row-blocks, each output column h'
# owned by exactly one block (all 7 reflect-folded taps inside it).
P4_STARTS = [0, 122, 244, 366, 480]   # block first row
P4_K = [128, 128, 128, 128, 32]       # block row count (partitions)
P4_WIN = [(0, 125), (125, 247), (247, 369), (369, 491), (491, 512)]
# xv>=5: all w-blocks exactly 128 wide (FWL needs 128-col stationaries;
# K<128 matmuls measured ~2-4x slower per instruction).  Extra rows in
# each block carry zero weights in bh, so results are unchanged.
WBLK128 = [(0, 128), (119, 247), (241, 369), (363, 491), (384, 512)]

_STATE: dict = {}
LDW_OPT = False  # walrus ldw-opt pass is broken (codegen rejects standalone LDW)


def _patch_ldw():
    """Flip --enable-ldw-opt on the walrus command line (process-local)."""
    if not LDW_OPT or _STATE.get("ldw_patched"):
        return
    from concourse import bass_utils as _bu
    orig = _bu.run_command

    def run_command(argv, **kwargs):
        argv = ["--enable-ldw-opt=true" if a == "--enable-ldw-opt=false"
                else a for a in argv]
        return orig(argv, **kwargs)

    _bu.run_command = run_command
    _STATE["ldw_patched"] = True


def _band_matrix() -> np.ndarray:
    """B[i, j] = multiplicity of input row i among the 7 reflect-padded taps
    of output row j."""
    B = np.zeros((512, 512), np.float32)
    j = np.arange(512)
    for d in range(-3, 4):
        i = np.abs(j + d)
        i = np.where(i > 511, 1022 - i, i)
        np.add.at(B, (i, j), 1.0)
    return B


def _build_consts(ilv: int = 4, xv: int = 0) -> dict:
    import ml_dtypes
    B = _band_matrix()
    if ilv == 4:
        # pass-1 rhs: column j holds B[S_k(j) + p, j] for its owner block
        f8 = ml_dtypes.float8_e4m3
        bvp = np.zeros((128, 512), f8 if xv == 7 else np.float16)
        for S, K, (a, b) in zip(P4_STARTS, P4_K, P4_WIN):
            bvp[:K, a:b] = B[S:S + K, a:b]
        consts = {"bvp": bvp}
    elif ilv in (2, 3):
        # pass-1 rhs chunks packed tight in R2_P1_MMS order; block (s, r)
        # partition p maps to input row 256*s + 2p + r.
        width = sum(b - a for (_, _, a, b, _, _) in R2_P1_MMS)
        bv = np.zeros((128, width), np.float16)
        off = 0
        for (s, r, a, b, _, _) in R2_P1_MMS:
            bv[:, off:off + b - a] = B[256 * s + r: 256 * (s + 1): 2, a:b]
            off += b - a
        consts = {"bv": bv}
    else:
        # pass-1 rhs: for each (block b, window): rows 128b..128b+128 of B,
        # cols P1_WINS[win], padded to width P1_STRIDE, laid side by side.
        bv = np.zeros((128, P1_STRIDE * len(P1_MMS)), np.float16)
        for k, (b, win) in enumerate(P1_MMS):
            s, e = P1_WINS[win]
            bv[:, P1_STRIDE * k: P1_STRIDE * k + (e - s)] = \
                B[128 * b: 128 * (b + 1), s:e]
        consts = {"bv": bv}
    # pass-2 rhs: for window j, rows are remapped to block j's partition
    # space (partition p = global w row WBLK[j][0] + p), cols WINS[j].
    wblk = WBLK128 if xv >= 5 else WBLK
    bh = np.zeros((128, 512), np.float16)
    for jw, ((ws, we), (s, e)) in enumerate(zip(wblk, WINS)):
        bh[: we - ws, s:e] = B[ws:we, s:e]
    consts["bh"] = bh
    return consts


def _permute_in(xs: np.ndarray, fp8: bool = False
                ) -> tuple[np.ndarray, np.ndarray]:
    """[n, 512, 512] row-major -> (xm [n,128,4,512], xtl [n,32,512]).
    fp8: quantize x-0.5 to e4m3 (the +0.5 is restored on-device in the
    output bias), halving input bytes at ~7e-3 rel err."""
    import ml_dtypes
    dt = ml_dtypes.float8_e4m3 if fp8 else np.float16
    if fp8:
        xs = xs - np.float32(0.5)
    xm = np.stack([xs[:, S:S + 128] for S in P4_STARTS[:4]], axis=2)
    xtl = xs[:, 480:512]
    return (np.ascontiguousarray(xm).astype(dt),
            np.ascontiguousarray(xtl).astype(dt))


def _unpermute_out(o: np.ndarray) -> np.ndarray:
    """Device output -> [n, 512, 512] natural layout.
    [n,128,4,512] chunk layout (row 128c+p at [:, p, c]) or [n,512,512]
    transposed (xv=6)."""
    if o.ndim == 4:
        return o.transpose(0, 2, 1, 3).reshape(o.shape[0], 512, 512)
    return np.ascontiguousarray(o.transpose(0, 2, 1))


def _build_nc(repeat: int = 1, loop_repeat: int = 0, group: int = -1,
              variant: str = "full", load_eng: str = "gp",
              store_eng: str = "sp", balance: int = 1, deep: int = 0,
              ilv: int = 4, pb: int = 44, xv: int = 5):
    """loop_repeat > 0 wraps the whole 12-image pipeline in a runtime
    For_i loop executing it that many times -- used only for timing (one
    NEFF dispatch, loop_repeat x the device work).
    variant: 'full' | 'dma' (loads+stores only) | 'nostore' (loads+compute)
    | 'load' (loads only) | 'store' (stores only) -- diagnostic builds.
    load_eng / store_eng: which queue issues the transfer --
    'gp' (SWDGE), 'sp' (qSP HWDGE), 'act' (qActivation HWDGE)."""
    _patch_ldw()
    do_dma = variant in ("full", "dma", "nostore", "load")
    do_compute = variant in ("full", "nostore", "comp")
    do_store = variant in ("full", "dma", "store")
    import concourse.tile as tile
    from concourse import bacc, mybir

    f16 = mybir.dt.float16
    f32 = mybir.dt.float32
    # xv=7: fp8 e4m3 input path (host sends x-0.5 quantized; +0.5 restored
    # in the output scale bias); halves input HBM traffic
    fx = mybir.dt.float8e4 if (ilv == 4 and xv == 7) else f16

    nc = bacc.Bacc("TRN2", target_bir_lowering=False, debug=False,
                   enable_asserts=True)
    IW = 5 * W if ilv == 4 else 4 * W   # per-image SBUF input width
    OW = 5 * W if (ilv == 4 and xv == 6) else 4 * W  # per-image output width

    if ilv == 4:
        xm_ap = nc.dram_tensor("xm", [IMGS, 128, 4, W], fx,
                               kind="ExternalInput").ap()
        xtl_ap = nc.dram_tensor("xtl", [IMGS, 32, W], fx,
                                kind="ExternalInput").ap()
        bvp_ap = nc.dram_tensor("bvp", [128, 512], fx,
                                kind="ExternalInput").ap()
        if xv == 6:
            # transposed output [i, w', h'] -- host untransposes
            out_ap = nc.dram_tensor("out", [IMGS, W, H], f16,
                                    kind="ExternalOutput").ap()
        else:
            out_ap = nc.dram_tensor("out", [IMGS, 128, 4, W], f16,
                                    kind="ExternalOutput").ap()
    else:
        bv_cols = (sum(b - a for (_, _, a, b, _, _) in R2_P1_MMS)
                   if ilv in (2, 3) else P1_STRIDE * len(P1_MMS))
        x_ap = nc.dram_tensor("x", [IMGS, H, W], f16,
                              kind="ExternalInput").ap()
        bv_ap = nc.dram_tensor("bv", [128, bv_cols], f16,
                               kind="ExternalInput").ap()
        out_ap = nc.dram_tensor("out", [IMGS, H, W], f16,
                                kind="ExternalOutput").ap()
    bh_ap = nc.dram_tensor("bh", [128, 512], f16, kind="ExternalInput").ap()

    def io_view(ap, gstart, GROUP, r2):
        # HBM <-> SBUF view of a group of images; r2 pairs 2 consecutive
        # rows per partition, merged into one 1024-elem (2 KB) run so each
        # DMA descriptor covers both rows.
        if r2:
            return ap[gstart:gstart + GROUP].rearrange(
                "i (s p r) w -> p i s (r w)", s=2, p=128, r=2)
        return ap[gstart:gstart + GROUP].rearrange(
            "i (s p) w -> p i s w", p=128)

    def out_view4(gstart, GROUP):
        return out_ap[gstart:gstart + GROUP].rearrange(
            "i p c w -> p i (c w)")

    ld_r2 = ilv in (2, 3)   # input layout / pass-1 block structure
    st_r2 = ilv == 2        # output layout / pass-2 chunk structure

    def dma_eng(which):
        return {"gp": nc.gpsimd, "sp": nc.sync, "act": nc.scalar}[which]

    def split_engs(spec):
        return ((spec[:2], spec[2:]) if spec[:2] in ("gp", "sp")
                else (spec[:3], spec[3:]))

    with tile.TileContext(nc) as tc, ExitStack() as ctx:
        cpool = ctx.enter_context(tc.tile_pool(name="const", bufs=1))
        xpool = ctx.enter_context(
            tc.tile_pool(name="xin", bufs=5 if deep else 3))
        t1ppool = ctx.enter_context(
            tc.tile_pool(name="t1p", bufs=pb // 10, space="PSUM"))
        t1pool = ctx.enter_context(tc.tile_pool(name="t1", bufs=10))
        oppool = ctx.enter_context(
            tc.tile_pool(name="opsum", bufs=pb % 10, space="PSUM"))
        outpool = ctx.enter_context(
            tc.tile_pool(name="osb", bufs=4 if deep else 3))

        if ilv == 4:
            bv = cpool.tile([128, 512], fx)
            nc.sync.dma_start(bv[:], bvp_ap[:])
        else:
            bv = cpool.tile([128, bv_cols], f16)
            nc.sync.dma_start(bv[:], bv_ap[:])
        bh = cpool.tile([128, 512], f16)
        nc.sync.dma_start(bh[:], bh_ap[:])

        dummy_osb = None
        if variant in ("dma", "store"):
            dummy_osb = cpool.tile([128, group * OW], f16)
            nc.vector.memset(dummy_osb[:], 0.0)
        dummy_x = None
        if variant == "comp":
            # compute-only: all groups read one zeroed const tile
            dummy_x = cpool.tile([128, max(2, abs(group)) * IW], fx)
            nc.vector.memset(dummy_x[:], 0.0)
        loop_ctx = (tc.For_i(0, loop_repeat, 1,
                             hint_engines=(mybir.EngineType.PE,))
                    if loop_repeat > 0 else None)
        if loop_ctx is not None:
            ctx.enter_context(loop_ctx)
        if group == 0:
            sizes = [1, 1, 2, 2, 2, 2, 1, 1]
        elif group == -1:
            sizes = [1, 1, 1, 2, 2, 2, 1, 1, 1]
        else:
            sizes = [group] * (IMGS // group)
        assert sum(sizes) == IMGS
        sched = []
        for rep in range(repeat):
            s0 = 0
            for gnum, sz in enumerate(sizes):
                for gi in range(sz):
                    sched.append((s0, sz, gi, gnum))
                s0 += sz
        ROT = ["gp", "sp", "act"]

        def load_group(xtg, gstart, GROUP, gnum):
            if ilv == 4:
                xgv = xtg.rearrange("p (i f) -> p i f", f=IW)
                dstm = xgv[:, :, :4 * W]
                dstt = xgv[:32, :, 4 * W:]
                srcm = xm_ap[gstart:gstart + GROUP].rearrange(
                    "i p b w -> p i (b w)")
                srct = xtl_ap[gstart:gstart + GROUP].rearrange(
                    "i p w -> p i w")
                if load_eng == "rot":
                    e = dma_eng(ROT[gnum % 3])
                    e.dma_start(dstm, srcm)
                    e.dma_start(dstt, srct)
                elif len(load_eng) > 3:
                    e0, e1 = split_engs(load_eng)
                    dma_eng(e0).dma_start(dstm[:64], srcm[:64])
                    dma_eng(e1).dma_start(dstm[64:], srcm[64:])
                    dma_eng(e1).dma_start(dstt, srct)
                else:
                    e = dma_eng(load_eng)
                    e.dma_start(dstm, srcm)
                    e.dma_start(dstt, srct)
                return
            src = io_view(x_ap, gstart, GROUP, ld_r2)
            if load_eng == "rot":
                dma_eng(ROT[gnum % 3]).dma_start(xtg[:], src)
            elif len(load_eng) > 3:  # e.g. 'gpsp', 'gpact', 'spact'
                e0, e1 = split_engs(load_eng)
                smid = 1 if ld_r2 else 2
                for i in range(GROUP):
                    dma_eng(e0).dma_start(
                        xtg[:, i * 4 * W: i * 4 * W + 2 * W],
                        src[:, i, :smid])
                    dma_eng(e1).dma_start(
                        xtg[:, i * 4 * W + 2 * W: (i + 1) * 4 * W],
                        src[:, i, smid:])
            else:
                dma_eng(load_eng).dma_start(xtg[:], src)

        def store_group6(osbg, gstart, GROUP, gnum):
            # transposed store: per w'-window jw, partitions = w' - s
            if store_eng == "rot":
                engs = [dma_eng(ROT[(gnum + 1 + jw) % 3]) for jw in range(5)]
            elif len(store_eng) > 3:
                e0, e1 = split_engs(store_eng)
                engs = [dma_eng(e0), dma_eng(e1), dma_eng(e0), dma_eng(e1),
                        dma_eng(e0)]
            else:
                engs = [dma_eng(store_eng)] * 5
            ov = osbg.rearrange("p (i f) -> p i f", f=5 * W)
            for jw, (s, e) in enumerate(WINS):
                dst = out_ap[gstart:gstart + GROUP, s:e].rearrange(
                    "i p h -> p i h")
                engs[jw].dma_start(dst, ov[:e - s, :, jw * W:(jw + 1) * W])

        def store_group(osbg, gstart, GROUP, gnum):
            if ilv == 4 and xv == 6:
                store_group6(osbg, gstart, GROUP, gnum)
                return
            out_view = (out_view4(gstart, GROUP) if ilv == 4
                        else io_view(out_ap, gstart, GROUP, st_r2))
            if store_eng == "rot":
                dma_eng(ROT[(gnum + 1) % 3]).dma_start(
                    out_view, osbg[:, :GROUP * 4 * W])
            elif len(store_eng) > 3:
                e0, e1 = split_engs(store_eng)
                if ilv == 4:
                    ov = out_view
                    src = osbg[:, :GROUP * 4 * W]
                    dma_eng(e0).dma_start(ov[:64], src[:64])
                    dma_eng(e1).dma_start(ov[64:], src[64:])
                else:
                    smid = 1 if st_r2 else 2
                    for i in range(GROUP):
                        dma_eng(e0).dma_start(
                            out_view[:, i, :smid],
                            osbg[:, i * 4 * W: i * 4 * W + 2 * W])
                        dma_eng(e1).dma_start(
                            out_view[:, i, smid:],
                            osbg[:, i * 4 * W + 2 * W: (i + 1) * 4 * W])
            else:
                dma_eng(store_eng).dma_start(
                    out_view, osbg[:, :GROUP * 4 * W])

        for (gstart, GROUP, g, gnum) in sched:
            img = gstart + g
            if g == 0:
                # load one image group; ilv=4 layout:
                # xtg[:, IW*i + 512*b + w] = block b of image gstart+i
                if variant == "comp":
                    xtg = dummy_x[:, :GROUP * IW]
                else:
                    xtg = xpool.tile([128, max(2, abs(group)) * IW], fx,
                                     tag="xt")
                    xtg = xtg[:, :GROUP * IW]
                if do_dma:
                    load_group(xtg, gstart, GROUP, gnum)
                if ilv == 4 and xv >= 5 and do_compute and variant != "comp":
                    # zero-pad the 24-row tail block to K=128 so its mms
                    # run as regular (128,128) tiles (K<128 mms are slow)
                    for i in range(GROUP):
                        eng = nc.vector if (gstart + i) % 2 == 0 else nc.gpsimd
                        # engine partition windows must be aligned pow2 blocks
                        eng.memset(
                            xtg[32:64, i * IW + 4 * W:(i + 1) * IW], 0.0)
                        eng.memset(
                            xtg[64:, i * IW + 4 * W:(i + 1) * IW], 0.0)
            xt = xtg[:, g * IW:(g + 1) * IW]
            if not do_compute:
                if g == GROUP - 1 and do_store:
                    store_group(dummy_osb, gstart, GROUP, gnum)
                continue

            # pass 1: T1[w, h'] per overlapping w-block j
            t1_tiles = []
            if ilv == 4 and xv == 4:
                # batched-tail order: all 5 K=24 tail mms first (one PE
                # tile-size reconfig pair per image, not per set)
                t1ps = []
                for jw, (ws, we) in enumerate(WBLK):
                    mj = we - ws
                    t1p = t1ppool.tile([128, 512], f32, tag="t1p")
                    a, b = P4_WIN[4]
                    nc.tensor.matmul(
                        t1p[:mj, a:b],
                        lhsT=xt[:P4_K[4], 4 * W + ws: 4 * W + we],
                        rhs=bv[:P4_K[4], a:b],
                        start=True, stop=False,
                    )
                    t1ps.append(t1p)
                for jw, (ws, we) in enumerate(WBLK):
                    mj = we - ws
                    t1p = t1ps[jw]
                    for k in range(4):
                        a, b = P4_WIN[k]
                        nc.tensor.matmul(
                            t1p[:mj, a:b],
                            lhsT=xt[:128, k * W + ws: k * W + we],
                            rhs=bv[:128, a:b],
                            start=False, stop=(k == 3),
                        )
                    t1 = t1pool.tile([128, 512], f16, tag="t1")
                    if balance == 3 and (jw + img) % 3 == 2:
                        nc.gpsimd.tensor_scalar_mul(t1[:mj, :], t1p[:mj, :], 1.0)
                    elif balance == 3 and (jw + img) % 3 == 0:
                        nc.vector.tensor_scalar_mul(t1[:mj, :], t1p[:mj, :], 1.0)
                    elif balance and (jw + img) % 2 == 0:
                        nc.vector.tensor_scalar_mul(t1[:mj, :], t1p[:mj, :], 1.0)
                    else:
                        nc.scalar.copy(t1[:mj, :], t1p[:mj, :])
                    t1_tiles.append((t1, mj))
            wblk = WBLK128 if (ilv == 4 and xv >= 5) else WBLK
            for jw, (ws, we) in (
                    [] if (ilv == 4 and xv == 4) else list(enumerate(wblk))):
                mj = we - ws
                t1p = t1ppool.tile([128, 512], f32, tag="t1p")
                if ilv == 4:
                    nk = 4 if xv in (1, 3) else 5
                    for k in range(nk):
                        a, b = P4_WIN[k]
                        K = 128 if xv >= 5 else P4_K[k]
                        nc.tensor.matmul(
                            t1p[:mj, a:b],
                            lhsT=xt[:K, k * W + ws: k * W + we],
                            rhs=bv[:K, a:b],
                            start=(k == 0), stop=(k == nk - 1),
                        )
                elif ld_r2:
                    off = 0
                    for (s, r, a, b, st, sp) in R2_P1_MMS:
                        nc.tensor.matmul(
                            t1p[:mj, a:b],
                            lhsT=xt[:, (s * 2 + r) * W + ws:
                                    (s * 2 + r) * W + we],
                            rhs=bv[:, off:off + b - a],
                            start=st, stop=sp,
                            # the boundary-strip mms re-open [253,259) to
                            # accumulate the s=1 rows onto the closed s=0
                            # group; plain PSUM read-modify-write on HW
                            skip_group_check=(a, b) == (253, 259) and not st,
                        )
                        off += b - a
                else:
                    for k, (b, win) in enumerate(P1_MMS):
                        s, e = P1_WINS[win]
                        nc.tensor.matmul(
                            t1p[:mj, s:e],
                            lhsT=xt[:, b * W + ws: b * W + we],
                            rhs=bv[:, P1_STRIDE * k: P1_STRIDE * k + (e - s)],
                            start=(k == 0), stop=(k == len(P1_MMS) - 1),
                        )
                t1 = t1pool.tile([128, 512], f16, tag="t1")
                # PSUM->SBUF fp16 cast, alternating ACT/DVE to balance load
                if balance == 3 and (jw + img) % 3 == 2:
                    nc.gpsimd.tensor_scalar_mul(t1[:mj, :], t1p[:mj, :], 1.0)
                elif balance == 3 and (jw + img) % 3 == 0:
                    nc.vector.tensor_scalar_mul(t1[:mj, :], t1p[:mj, :], 1.0)
                elif balance and (jw + img) % 2 == 0:
                    nc.vector.tensor_scalar_mul(t1[:mj, :], t1p[:mj, :], 1.0)
                else:
                    nc.scalar.copy(t1[:mj, :], t1p[:mj, :])
                t1_tiles.append((t1, mj))

            # pass 2: O[h', w'] per 128-row h' chunk c
            if g == 0:
                osbg = outpool.tile([128, max(2, abs(group)) * OW], f16,
                                    tag="osb")
                osbg = osbg[:, :GROUP * OW]
            osb = osbg[:, g * OW:(g + 1) * OW]
            if ilv == 4 and xv == 6:
                # flipped pass 2: stationary = bh block, moving = full t1;
                # one N=512 matmul per w-block, output transposed [w', h']
                for jw, (t1, mj) in enumerate(t1_tiles):
                    s, e = WINS[jw]
                    opt = oppool.tile([128, 512], f32, tag="op")
                    nc.tensor.matmul(
                        opt[:e - s, :],
                        lhsT=bh[:mj, s:e],
                        rhs=t1[:mj, :],
                        start=True, stop=True,
                    )
                    dst = osb[:, jw * W:(jw + 1) * W]
                    if (jw + img) % 2 == 0:
                        nc.vector.tensor_scalar_mul(
                            dst[:e - s], opt[:e - s, :], 1.0 / 49.0)
                    else:
                        nc.scalar.mul(dst[:e - s], opt[:e - s, :], 1.0 / 49.0)
                if g == GROUP - 1 and do_store:
                    store_group(osbg, gstart, GROUP, gnum)
                continue
            for c in range(4):
                op = oppool.tile([128, 512], f32, tag="op")
                p2 = t1_tiles[:4] if xv in (2, 3) else t1_tiles
                for jw, (t1, mj) in enumerate(p2):
                    s, e = WINS[jw]
                    if st_r2:
                        # chunk c=(sc, rc): output rows 256*sc + 2p + rc,
                        # i.e. t1 cols strided 2 starting at 256*sc + rc
                        sc, rc = R2_CHUNKS[c]
                        lhsT = t1[:mj].rearrange(
                            "q (s p r) -> q s r p", s=2, p=128, r=2)[:, sc, rc, :]
                    else:
                        lhsT = t1[:mj, c * 128: (c + 1) * 128]
                    nc.tensor.matmul(
                        op[:, s:e],
                        lhsT=lhsT,
                        rhs=bh[:mj, s:e],
                        start=(jw == 0), stop=(jw == len(p2) - 1),
                    )
                # final 1/49 scale + PSUM->SBUF fp16, split across DVE and ACT
                dst = osb[:, c * W: (c + 1) * W]
                bias = 0.5 if xv == 7 else 0.0
                if c % 2 == 0:
                    if bias:
                        nc.vector.tensor_scalar(
                            dst, op[:], 1.0 / 49.0, bias,
                            mybir.AluOpType.mult, mybir.AluOpType.add)
                    else:
                        nc.vector.tensor_scalar_mul(dst, op[:], 1.0 / 49.0)
                else:
                    nc.scalar.activation(
                        dst, op[:], mybir.ActivationFunctionType.Copy,
                        bias=bias, scale=1.0 / 49.0)
            if g == GROUP - 1 and do_store:
                store_group(osbg, gstart, GROUP, gnum)

    nc.compile()
    return nc


def _get_state(repeat: int = 1, loop_repeat: int = 0, group: int = -1,
               variant: str = "full", load_eng: str = "gp",
               store_eng: str = "sp", balance: int = 1, deep: int = 0,
               ilv: int = 4, pb: int = 44, xv: int = 5):
    key = ("nc", repeat, loop_repeat, group, variant, load_eng, store_eng,
           balance, deep, ilv, pb, xv)
    if key not in _STATE:
        _STATE[key] = _build_nc(repeat, loop_repeat, group, variant,
                                load_eng, store_eng, balance, deep, ilv, pb,
                                xv)
    ckey = ("consts", ilv, xv)
    if ckey not in _STATE:
        _STATE[ckey] = _build_consts(ilv, xv)
    return {"nc": _STATE[key], "consts": _STATE[ckey], "ilv": ilv, "xv": xv}


def _make_runner(repeat: int = 1, loop_repeat: int = 0, group: int = -1,
                 variant: str = "full", load_eng: str = "gp",
                 store_eng: str = "sp", balance: int = 1, deep: int = 0,
                 ilv: int = 4, pb: int = 44, xv: int = 5):
    """Cached 8-core sharded jit over the bass program (mirrors
    bass2jax.run_bass_via_pjrt's multicore path, minus buffer donation so
    the compiled fn can be invoked repeatedly for timing)."""
    rkey = ("runner", repeat, loop_repeat, group, variant, load_eng,
            store_eng, balance, deep, ilv, pb, xv)
    if rkey in _STATE:
        return _STATE[rkey]
    import jax
    import jax.numpy as jnp
    from jax.sharding import Mesh, PartitionSpec
    from jax.experimental.shard_map import shard_map
    from concourse import bass2jax, mybir

    st = _get_state(repeat, loop_repeat, group, variant, load_eng,
                    store_eng, balance, deep, ilv, pb, xv)
    nc = st["nc"]
    bass2jax.install_neuronx_cc_hook()

    partition_name = (nc.partition_id_tensor.name
                      if nc.partition_id_tensor else None)
    in_names, out_names, out_avals = [], [], []
    for alloc in nc.m.functions[0].allocations:
        if not isinstance(alloc, mybir.MemoryLocationSet):
            continue
        name = alloc.memorylocations[0].name
        if alloc.kind == "ExternalInput":
            if name != partition_name:
                in_names.append(name)
        elif alloc.kind == "ExternalOutput":
            out_names.append(name)
            out_avals.append(jax.core.ShapedArray(
                tuple(alloc.tensor_shape), mybir.dt.np(alloc.dtype)))
    n_params = len(in_names)
    all_names = in_names + out_names
    if partition_name is not None:
        all_names = all_names + [partition_name]

    def _body(*args):
        operands = list(args)
        if partition_name is not None:
            operands.append(bass2jax.partition_id_tensor())
        outs = bass2jax._bass_exec_p.bind(
            *operands,
            out_avals=tuple(out_avals),
            in_names=tuple(all_names),
            out_names=tuple(out_names),
            lowering_input_output_aliases=(),
            sim_require_finite=True,
            sim_require_nnan=True,
            nc=nc,
        )
        return tuple(outs)

    devices = jax.devices()[:N_CORES]
    mesh = Mesh(np.asarray(devices), ("core",))
    n_outs = len(out_names)
    sharded = jax.jit(shard_map(
        _body, mesh=mesh,
        in_specs=(PartitionSpec("core"),) * (n_params + n_outs),
        out_specs=(PartitionSpec("core"),) * n_outs,
        check_rep=False))
    _STATE[rkey] = (sharded, in_names, out_names, out_avals)
    return _STATE[rkey]


def _core_inputs(x: np.ndarray, ilv: int, consts: dict,
                 xv: int = 0) -> list[dict]:
    """Per-core input dicts from the full fp32 batch."""
    B, C = x.shape[0], x.shape[1]
    per = B // N_CORES
    maps = []
    for i in range(N_CORES):
        xs = np.ascontiguousarray(
            x[i * per:(i + 1) * per].reshape(per * C, H, W))
        if ilv == 4:
            xm, xtl = _permute_in(xs, fp8=(xv == 7))
            m = {"xm": xm, "xtl": xtl}
        else:
            m = {"x": xs.astype(np.float16)}
        m.update(consts)
        maps.append(m)
    return maps


def _concat_inputs(x: np.ndarray, **kw):
    st = _get_state(**kw)
    _, in_names, out_names, out_avals = _make_runner(**kw)
    maps = _core_inputs(np.asarray(x, np.float32), st["ilv"], st["consts"],
                        st["xv"])
    concat_in = []
    for n in in_names:
        a = np.stack([m[n] for m in maps])
        concat_in.append(np.ascontiguousarray(
            a.reshape((N_CORES * a.shape[1],) + a.shape[2:])))
    concat_zeros = [
        np.zeros((N_CORES * a.shape[0],) + a.shape[1:], a.dtype)
        for a in out_avals]
    return concat_in, concat_zeros


def kernel(x: np.ndarray) -> np.ndarray:
    _patch_ldw()
    from concourse import bass_utils
    st = _get_state()
    x = np.asarray(x, np.float32)
    B, C = x.shape[0], x.shape[1]
    per = B // N_CORES
    in_maps = _core_inputs(x, st["ilv"], st["consts"], st["xv"])
    res = bass_utils.run_bass_kernel_spmd(
        st["nc"], in_maps, core_ids=list(range(N_CORES)))
    outs = []
    for i in range(N_CORES):
        o = res.results[i]["out"]
        if st["ilv"] == 4:
            o = _unpermute_out(np.asarray(o))
        outs.append(o.reshape(per, C, H, W))
    out = np.concatenate(outs, axis=0)
    return np.ascontiguousarray(out).astype(np.float32)


def benchmark(x: np.ndarray, iters: int = 30) -> float:
    """Returns steady-state per-invocation wall time in ns for the 8-core
    SPMD execution (inputs sharded and resident on their devices; outputs
    chained into the next call's scratch operand so iterations pipeline
    without host round-trips)."""
    import time
    import jax
    from jax.sharding import Mesh, NamedSharding, PartitionSpec

    x = np.asarray(x, np.float32)
    sharded, in_names, out_names, out_avals = _make_runner()
    concat_in, concat_zeros = _concat_inputs(x)
    devices = jax.devices()[:N_CORES]
    mesh = Mesh(np.asarray(devices), ("core",))
    shard0 = NamedSharding(mesh, PartitionSpec("core"))
    dev_in = [jax.device_put(a, shard0) for a in concat_in]
    dev_zero = [jax.device_put(a, shard0) for a in concat_zeros]
    # warm up (compiles on first call)
    outs = sharded(*dev_in, *dev_zero)
    jax.block_until_ready(outs)
    # chained steady-state loop: prior outputs feed the scratch-out slots
    t0 = time.perf_counter()
    for _ in range(iters):
        outs = sharded(*dev_in, *outs)
    jax.block_until_ready(outs)
    dt = (time.perf_counter() - t0) / iters
    return dt * 1e9
